# revision 19
# baseline (speedup 1.0000x reference)
"""Distributed 2-layer GAT (BangaloreGAT) on 8 TRN2 NeuronCores.

Strategy: partition destination nodes across the 8 cores (6250 each). Host
routes every edge (incl. self-loops) to the core owning its dst, sorts by
dst, and packs edges into 49 dst-blocks x T tiles of 128 slots. Per layer
each core builds an fp16 gather table row [h | s] in its own DRAM (layer 1:
replicated compute from x; layer 2: own stripe + AllGather), then per
dst-block: batched indirect-DMA gathers of the edge sources' table rows
(1024 idx/call — the SWDGE ucode cap — spread round-robin over 4 SWDGE
queues so descriptor-ring prep/drain pipelines instead of serializing),
an indicator-matrix build (is_equal vs iota, fp16), and TensorE fp16
matmuls that segment-reduce alpha-weighted features and softmax
denominators in f32 PSUM. Per-edge dst terms come from a separate f32
d-table gather (merged across block pairs to halve SWDGE launches).
BatchNorm/bias are folded host-side into the table weights (W_ext also
carries the attention dot-product columns); ELU's "-1" is folded into the
next layer's bias columns. The softmax/ELU/output chains stay f32 — the
final fc reduction amplifies operand noise ~7x, so bf16 there fails the
2e-2 gate while fp16 tables + f32 post-chains hold rel err at 4.3e-3.
"""
import sys
from contextlib import ExitStack
import numpy as np
import ml_dtypes

sys.path.insert(0, '/opt/trn_rl_repo')
sys.path.insert(0, '/root/problem')

# ---------------- problem constants (hardcoded from the spec) --------------
N = 50000
E = 800000
FIN = 128
H = 4
C1 = 64
C2 = 32
D1 = H * C1            # 256
D2 = H * C2            # 128
R1 = D1 + H            # 260 table-1 row (h|s)
R2 = D2 + H            # 132 table-2 row
NCORES = 8
NO = N // NCORES       # 6250 owned dst nodes / core
P = 128
NB = 49                # dst blocks per core: ceil(6250/128)
NOP = NB * P           # padded own nodes (6272)
NPAD = ((N + P - 1) // P) * P + P  # padded table-1 rows (50048+)
EPS_BN = 1e-5


def _multiwait_fix(nc):
    from concourse import mybir
    cnt = [0]
    for f in nc.m.functions:
        for bb in f.blocks:
            insts = bb.instructions
            new = []
            changed = False
            for inst in insts:
                si = getattr(inst, "sync_info", None)
                waits = list(si.on_wait) if si is not None else []
                if len(waits) > 1:
                    changed = True
                    for w in waits[:-1]:
                        cnt[0] += 1
                        nop = mybir.InstNoOp(name=f"I-ws{cnt[0]}", ins=[], outs=[])
                        nop.engine = inst.engine
                        nop.sync_info = mybir.SyncInfo(on_wait=[w], on_update=[])
                        new.append(nop)
                    si.on_wait = [waits[-1]]
                new.append(inst)
            if changed:
                bb.instructions = new


# ---------------------------- host preprocessing ---------------------------
def preprocess(x, edge_index, W1, a1_src, a1_dst, b1, g1, be1, m1, v1,
               W2, a2_src, a2_dst, b2, g2, be2, m2, v2, fcW, fcb):
    x = np.asarray(x, np.float32)
    ei = np.asarray(edge_index)
    src = np.concatenate([ei[0], np.arange(N, dtype=np.int64)]).astype(np.int64)
    dst = np.concatenate([ei[1], np.arange(N, dtype=np.int64)]).astype(np.int64)

    # folded weights
    W1 = np.asarray(W1, np.float32); W2 = np.asarray(W2, np.float32)
    a1_src = np.asarray(a1_src, np.float32); a1_dst = np.asarray(a1_dst, np.float32)
    a2_src = np.asarray(a2_src, np.float32); a2_dst = np.asarray(a2_dst, np.float32)
    g1 = np.asarray(g1, np.float32); be1 = np.asarray(be1, np.float32)
    m1 = np.asarray(m1, np.float32); v1 = np.asarray(v1, np.float32)
    g2 = np.asarray(g2, np.float32); be2 = np.asarray(be2, np.float32)
    m2 = np.asarray(m2, np.float32); v2 = np.asarray(v2, np.float32)
    b1 = np.asarray(b1, np.float32); b2 = np.asarray(b2, np.float32)
    fcW = np.asarray(fcW, np.float32); fcb = np.asarray(fcb, np.float32)

    scale1 = g1 / np.sqrt(v1 + EPS_BN)
    shift1 = be1 - m1 * scale1
    shtot1 = scale1 * b1 + shift1                      # [256]
    W1p = W1 * scale1[None, :]                         # [128,256]
    w_s1 = np.einsum('fhc,hc->fh', W1.reshape(FIN, H, C1), a1_src)  # [128,4]
    w_d1 = np.einsum('fhc,hc->fh', W1.reshape(FIN, H, C1), a1_dst)
    W1ext = np.concatenate([W1p, w_s1, w_d1], axis=1).astype(np.float32)  # [128,264]

    scale2 = g2 / np.sqrt(v2 + EPS_BN)
    shift2 = be2 - m2 * scale2
    shtot2 = scale2 * b2 + shift2                      # [128]
    W2p = W2 * scale2[None, :]                         # [256,128]
    w_s2 = np.einsum('fhc,hc->fh', W2.reshape(D1, H, C2), a2_src)   # [256,4]
    w_d2 = np.einsum('fhc,hc->fh', W2.reshape(D1, H, C2), a2_dst)
    W2ext = np.concatenate([W2p, w_s2, w_d2], axis=1).astype(np.float32)  # [256,136]
    c2 = -W2ext.sum(axis=0).astype(np.float32)         # [136] (x2 = u-1 fold)
    cfc = np.float32(fcb[0] - fcW.sum())
    fcw_row = fcW.reshape(1, D2).astype(np.float32)

    # --- edge routing: per core, sort by dst, pack into blocks ---
    owner = dst // NO
    per_core = []
    maxT = 1
    for c in range(NCORES):
        m = owner == c
        s_c = src[m]
        dl = (dst[m] - c * NO).astype(np.int64)        # [0, 6250)
        order = np.argsort(dl, kind='stable')
        s_c = s_c[order]; dl = dl[order]
        blk = dl // P
        cnt = np.bincount(blk, minlength=NB)
        maxT = max(maxT, int(np.ceil(cnt.max() / P)))
        per_core.append((s_c, dl, blk, cnt))

    # per-layer A/B split (int16 gather limit 32768): recompute tile counts
    SPLIT = 32768
    def pack_layer(per_core, rowfn):
        # returns per-core slot arrays + (TA, TB)
        TA = TB = 1
        packed = []
        for c in range(NCORES):
            s_c, dl, blk, cnt = per_core[c]
            rows = rowfn(s_c, c)
            blocks = []
            start = 0
            for b in range(NB):
                n_b = int(cnt[b])
                sl = slice(start, start + n_b)
                r = rows[sl]; d = dl[sl]
                isa = r < SPLIT
                blocks.append((r[isa], d[isa], r[~isa], d[~isa]))
                TA = max(TA, int(np.ceil(len(blocks[-1][0]) / P)))
                TB = max(TB, int(np.ceil(len(blocks[-1][2]) / P)))
                start += n_b
            packed.append(blocks)
        return packed, TA, TB

    rf1 = lambda s, c: ((s - c * NO) % N).astype(np.int32)
    rf2 = lambda s, c: ((s // NO) * NOP + (s % NO)).astype(np.int32)
    packed1, TA1, TB1 = pack_layer(per_core, rf1)
    packed2, TA2, TB2 = pack_layer(per_core, rf2)
    T1g, T2g = TA1 + TB1, TA2 + TB2

    def build_arrays(blocks, TA, TB):
        T = TA + TB
        gidx = np.zeros((NB, T * P), np.int32)
        dloc = np.full((NB, T * P), 999.0, np.float32)
        for b, (ra, da, rb, db) in enumerate(blocks):
            na, nb_ = len(ra), len(rb)
            gidx[b, :na] = ra
            dloc[b, :na] = (da - b * P).astype(np.float32)
            off = TA * P
            gidx[b, off:off + nb_] = rb - SPLIT
            dloc[b, off:off + nb_] = (db - b * P).astype(np.float32)
        ddid = np.where(dloc < 999.0, 0, 0).astype(np.int32)  # placeholder
        # d-table index = absolute local dst = dloc + 128*b (pad -> 0)
        ddid = np.zeros((NB, T * P), np.int32)
        for b in range(NB):
            v = dloc[b]
            ddid[b] = np.where(v < 999.0, v + b * P, 0).astype(np.int32)
        return gidx, dloc, ddid

    def wrap16(a):      # [NB, K] int -> wrapped int16 [128, NB*K//16]
        NBb, K = a.shape
        w = a.reshape(NBb, K // 16, 16).transpose(2, 0, 1).reshape(16, NBb * (K // 16))
        return np.tile(w, (8, 1)).astype(np.int16)

    def pm(a, T):
        return np.ascontiguousarray(
            a.reshape(NB, T, P).transpose(2, 0, 1).reshape(P, NB * T))

    in_maps = []
    for c in range(NCORES):
        g1, dl1, dd1 = build_arrays(packed1[c], TA1, TB1)
        g2, dl2, dd2 = build_arrays(packed2[c], TA2, TB2)
        xr = np.roll(x, -c * NO, axis=0)
        xT = np.zeros((FIN, NPAD), np.float32)
        xT[:, :N] = xr.T
        in_maps.append({
            "xT": xT.astype(np.float16),
            "g1w": wrap16(g1), "g2w": wrap16(g2),
            "dd1w": wrap16(dd1), "dd2w": wrap16(dd2),
            "dloc1": pm(dl1, T1g).astype(np.float16),
            "dloc2": pm(dl2, T2g).astype(np.float16),
            "W1ext": W1ext.astype(np.float16),
            "W2ext": W2ext.astype(np.float16),
            "sh1": np.tile(shtot1.reshape(1, D1), (P, 1)),
            "sh2": np.tile(shtot2.reshape(1, D2), (P, 1)),
            "c2": np.tile(c2.reshape(1, D2 + 2 * H), (P, 1)),
            "fcw": np.tile(fcw_row, (P, 1)),
        })
    return in_maps, (TA1, TB1, TA2, TB2), cfc
    return in_maps, T, cfc


# ------------------------------ bass builder -------------------------------
SPLIT = 32768
TW1 = 384   # table-1 row stride (fp16 elems, 768B)
TW2 = 256   # table-2 row stride (fp16 elems, 512B)
DW = 64     # d-table row stride (f32, 256B)
GNT = 8     # tiles (x128 idx) per dma_gather call (1024-idx ucode cap)


def build_module(Ts, cfc, reps=1, debug=False):
    from concourse import bass, mybir, bacc
    import concourse.tile as tile
    from concourse.masks import make_identity

    TA1, TB1, TA2, TB2 = Ts
    T1g, T2g = TA1 + TB1, TA2 + TB2
    f32 = mybir.dt.float32
    f32r = mybir.dt.float32r
    bf16 = mybir.dt.float16
    i16 = mybir.dt.int16
    AF = mybir.ActivationFunctionType
    OP = mybir.AluOpType

    nc = bacc.Bacc(dynamic_dma_scratch_size=32768, num_swdge_queues=4)
    xT_p = nc.declare_dram_parameter("xT", [FIN, NPAD], bf16, isOutput=False)
    g1w_p = nc.declare_dram_parameter("g1w", [P, NB * T1g * 8], i16, isOutput=False)
    g2w_p = nc.declare_dram_parameter("g2w", [P, NB * T2g * 8], i16, isOutput=False)
    dd1w_p = nc.declare_dram_parameter("dd1w", [P, NB * T1g * 8], i16, isOutput=False)
    dd2w_p = nc.declare_dram_parameter("dd2w", [P, NB * T2g * 8], i16, isOutput=False)
    dloc1_p = nc.declare_dram_parameter("dloc1", [P, NB * T1g], bf16, isOutput=False)
    dloc2_p = nc.declare_dram_parameter("dloc2", [P, NB * T2g], bf16, isOutput=False)
    W1e_p = nc.declare_dram_parameter("W1ext", [FIN, D1 + 2 * H], bf16, isOutput=False)
    W2e_p = nc.declare_dram_parameter("W2ext", [D1, D2 + 2 * H], bf16, isOutput=False)
    sh1_p = nc.declare_dram_parameter("sh1", [P, D1], f32, isOutput=False)
    sh2_p = nc.declare_dram_parameter("sh2", [P, D2], f32, isOutput=False)
    c2_p = nc.declare_dram_parameter("c2", [P, D2 + 2 * H], f32, isOutput=False)
    fcw_p = nc.declare_dram_parameter("fcw", [P, D2], f32, isOutput=False)
    out_p = nc.declare_dram_parameter("out", [NOP, 1], f32, isOutput=True)

    t1a = nc.dram_tensor("t1taba", [SPLIT, TW1], bf16)
    t1b = nc.dram_tensor("t1tabb", [NPAD - SPLIT, TW1], bf16)
    d1t = nc.dram_tensor("d1tab", [8192, DW], f32)
    t2own = nc.dram_tensor("t2own", [NOP, TW2], bf16)
    d2t = nc.dram_tensor("d2tab", [NOP, DW], f32)
    t2full = nc.dram_tensor("t2full", [NCORES * NOP, TW2], bf16, addr_space="Shared")

    def apx(base_ap, off, pattern):
        return bass.AP(tensor=base_ap.tensor, offset=base_ap.offset + off,
                       ap=[list(base_ap.ap[0])] + [list(q) for q in pattern])

    def load(tc_pool, nc, param, shape, dt, tag):
        t = tc_pool.tile(shape, dt, tag=tag)
        nc.sync.dma_start(out=t[:], in_=param[:])
        return t

    QCTR = [0]

    def nextq():
        QCTR[0] = (QCTR[0] + 1) % 4
        return QCTR[0]

    def gathers(nc, dest, widx_t, wcol0, n_tiles, tbl_ap, elem):
        # chunked dma_gather: <=1024 idx per call (SWDGE ring limit/queue)
        done = 0
        while done < n_tiles:
            nt = min(n_tiles - done, GNT)
            K = nt * P
            out_ap = apx(dest[:], (done) * elem, [[elem, nt], [1, elem]])
            nc.gpsimd.dma_gather(
                out_ap=out_ap, in_ap=tbl_ap,
                idxs_ap=widx_t[:, wcol0 + done * 8: wcol0 + (done + nt) * 8],
                num_idxs=K, num_idxs_reg=K, elem_size=elem, queue_num=nextq())
            done += nt

    NTILE1 = NPAD // P

    for rep in range(reps):
        with tile.TileContext(nc) as tc, ExitStack() as ctx:
            consts = ctx.enter_context(tc.tile_pool(name=f"consts{rep}", bufs=1))
            sbuf = ctx.enter_context(tc.tile_pool(name=f"sbuf{rep}", bufs=2))
            gp3 = ctx.enter_context(tc.tile_pool(name=f"gp3{rep}", bufs=3))
            sb3 = ctx.enter_context(tc.tile_pool(name=f"sb3{rep}", bufs=3))
            psA = ctx.enter_context(tc.tile_pool(name=f"psA{rep}", bufs=2, space="PSUM"))
            psB = ctx.enter_context(tc.tile_pool(name=f"psB{rep}", bufs=3, space="PSUM"))
            psC = ctx.enter_context(tc.tile_pool(name=f"psC{rep}", bufs=2, space="PSUM"))

            w1e = consts.tile([FIN, D1 + 2 * H], bf16)
            nc.sync.dma_start(out=w1e[:], in_=W1e_p[:])
            w2e = consts.tile([P, 2, D2 + 2 * H], bf16)
            nc.sync.dma_start(out=w2e[:, 0, :], in_=W2e_p[0:P, :])
            nc.sync.dma_start(out=w2e[:, 1, :], in_=W2e_p[P:2 * P, :])
            sh1r = load(consts, nc, sh1_p, [P, D1], f32, "sh1r")
            c2r = load(consts, nc, c2_p, [P, D2 + 2 * H], f32, "c2r")
            g1w_t = load(consts, nc, g1w_p, [P, NB * T1g * 8], i16, "g1w")
            dd1w_t = load(consts, nc, dd1w_p, [P, NB * T1g * 8], i16, "dd1w")
            dloc_t = load(consts, nc, dloc1_p, [P, NB * T1g], bf16, "dloc1")
            iota_i = consts.tile([P, P], mybir.dt.int32)
            nc.gpsimd.iota(iota_i[:], pattern=[[1, P]], base=0, channel_multiplier=0)
            iota_r = consts.tile([P, P], bf16)
            nc.vector.tensor_copy(iota_r[:], iota_i[:])
            ident = consts.tile([P, P], bf16)
            make_identity(nc, ident[:])

            # --- table-1 build (replicated; 8 tiles/iter; A region first) --
            W4 = 8
            JSPLIT = SPLIT // (W4 * P)  # iters 0..JSPLIT-1 -> t1a, rest -> t1b
            for j in range(NTILE1 // W4):
                xc = sb3.tile([FIN, W4 * P], bf16, tag="xc")
                nc.sync.dma_start(out=xc[:], in_=xT_p[:, j * W4 * P:(j + 1) * W4 * P])
                row = sb3.tile([P, W4, R1], bf16, tag="t1row")
                drow = sb3.tile([P, W4, H], f32, tag="t1d")
                for k in range(W4):
                    ps = psA.tile([P, D1 + 2 * H], f32, tag="t1ps")
                    nc.tensor.matmul(out=ps[:], lhsT=xc[:, k * P:(k + 1) * P],
                                     rhs=w1e[:], start=True, stop=True)
                    if k % 2 == 0:
                        nc.vector.tensor_copy(row[:, k, :], ps[:, 0:R1])
                    else:
                        nc.scalar.copy(row[:, k, :], ps[:, 0:R1])
                    if j * W4 + k < NB:
                        nc.vector.tensor_copy(drow[:, k, :], ps[:, R1:R1 + H])
                # one 3D DMA writes 8 consecutive row-tiles into its region
                tdst = t1a if j < JSPLIT else t1b
                joff = j if j < JSPLIT else j - JSPLIT
                tref = tdst[0:P, 0:R1]
                nc.sync.dma_start(
                    out=bass.AP(tensor=tref.tensor, offset=joff * W4 * P * TW1,
                                ap=[[TW1, P], [P * TW1, W4], [1, R1]]),
                    in_=row[:])
                if j * W4 < NB:
                    d1b = d1t[0:P, 0:H]
                    nc.sync.dma_start(
                        out=bass.AP(tensor=d1b.tensor, offset=j * W4 * P * DW,
                                    ap=[[DW, P], [P * DW, W4], [1, H]]),
                        in_=drow[:])

            # ---------------- layer-1 edge pass ----------------------------
            T = T1g
            for b in range(NB):
                G = gp3.tile([P, T * TW1], bf16, tag="G")
                gathers(nc, G, g1w_t, b * T * 8, TA1, t1a[:], TW1)
                # B-half gather (table view offset SPLIT rows)
                doneB = 0
                while doneB < TB1:
                    nt = min(TB1 - doneB, GNT)
                    K = nt * P
                    out_ap = apx(G[:], (TA1 + doneB) * TW1, [[TW1, nt], [1, TW1]])
                    nc.gpsimd.dma_gather(
                        out_ap=out_ap, in_ap=t1b[:],
                        idxs_ap=g1w_t[:, (b * T + TA1 + doneB) * 8:(b * T + TA1 + doneB + nt) * 8],
                        num_idxs=K, num_idxs_reg=K, elem_size=TW1, queue_num=nextq())
                    doneB += nt
                if b % 2 == 0:
                    npair = 2 * T if b + 1 < NB else T
                    DdP = sbuf.tile([P, 2 * T * DW], f32, tag="Dd")
                    gathers(nc, DdP, dd1w_t, b * T * 8, npair, d1t[:], DW)
                Dd = DdP[:, (b % 2) * T * DW:(b % 2 + 1) * T * DW]
                me = gp3.tile([P, T * P], bf16, tag="me")
                nc.vector.tensor_tensor(
                    out=apx(me[:], 0, [[P, T], [1, P]]),
                    in0=apx(dloc_t[:], b * T, [[1, T], [0, P]]),
                    in1=apx(iota_r[:], 0, [[0, T], [1, P]]),
                    op=OP.is_equal)
                st = sbuf.tile([P, T * H], f32, tag="st")
                nc.vector.tensor_tensor(
                    out=apx(st[:], 0, [[H, T], [1, H]]),
                    in0=apx(G[:], D1, [[TW1, T], [1, H]]),
                    in1=apx(Dd, 0, [[DW, T], [1, H]]), op=OP.add)
                st2 = sbuf.tile([P, T * H], f32, tag="st2")
                nc.vector.tensor_scalar_mul(st2[:], st[:], 0.2)
                nc.vector.tensor_tensor(out=st[:], in0=st[:], in1=st2[:], op=OP.max)
                ex = sbuf.tile([P, T * H], bf16, tag="ex")
                nc.scalar.activation(ex[:], st[:], AF.Exp)
                we = gp3.tile([P, T * R1], bf16, tag="we")
                nc.vector.tensor_tensor(
                    out=apx(we[:], 0, [[R1, T], [1, D1]]),
                    in0=apx(G[:], 0, [[TW1, T], [1, D1]]),
                    in1=apx(ex[:], 0, [[H, T], [1, H], [0, C1]]), op=OP.mult)
                nc.vector.tensor_copy(
                    out=apx(we[:], D1, [[R1, T], [1, H]]),
                    in_=apx(ex[:], 0, [[H, T], [1, H]]))
                po = psB.tile([P, R1], f32, tag="po")
                for t in range(T):
                    nc.tensor.matmul(
                        out=po[:],
                        lhsT=me[:, t * P:(t + 1) * P],
                        rhs=we[:, t * R1:(t + 1) * R1],
                        start=(t == 0), stop=(t == T - 1))
                rden = sbuf.tile([P, H], f32, tag="rden")
                nc.vector.reciprocal(rden[:], po[:, D1:D1 + H])
                r = sbuf.tile([P, D1], f32, tag="r")
                nc.vector.tensor_tensor(
                    out=apx(r[:], 0, [[C1, H], [1, C1]]),
                    in0=apx(po[:], 0, [[C1, H], [1, C1]]),
                    in1=apx(rden[:], 0, [[1, H], [0, C1]]), op=OP.mult)
                nc.vector.tensor_add(r[:], r[:], sh1r[:])
                tneg = sbuf.tile([P, D1], f32, tag="tneg")
                nc.vector.tensor_scalar_min(tneg[:], r[:], 0.0)
                texp = sbuf.tile([P, D1], f32, tag="texp")
                nc.scalar.activation(texp[:], tneg[:], AF.Exp)
                u = sbuf.tile([P, D1], bf16, tag="u")
                nc.vector.tensor_scalar_max(u[:], r[:], 0.0)
                nc.vector.tensor_add(u[:], u[:], texp[:])
                uT = sbuf.tile([P, 2, P], bf16, tag="uT")
                for k in range(2):
                    pt = psC.tile([P, P], bf16, tag="pt", bufs=1)
                    nc.tensor.transpose(out=pt[:], in_=u[:, k * P:(k + 1) * P],
                                        identity=ident[:])
                    nc.vector.tensor_copy(uT[:, k, :], pt[:])
                p2 = psC.tile([P, D2 + 2 * H], f32, tag="p2")
                for k in range(2):
                    nc.tensor.matmul(out=p2[:], lhsT=uT[:, k, :],
                                     rhs=w2e[:, k, :],
                                     start=(k == 0), stop=(k == 1))
                row2 = sbuf.tile([P, R2], bf16, tag="row2")
                nc.vector.tensor_add(row2[:], p2[:, 0:R2], c2r[:, 0:R2])
                nc.sync.dma_start(out=t2own[b * P:(b + 1) * P, 0:R2],
                                  in_=row2[:])
                d2row = sbuf.tile([P, H], f32, tag="d2row")
                nc.vector.tensor_add(d2row[:], p2[:, R2:R2 + H], c2r[:, R2:R2 + H])
                nc.sync.dma_start(out=d2t[b * P:(b + 1) * P, 0:H],
                                  in_=d2row[:])

            # -------- AllGather inside the same TileContext ---------------
            nc.gpsimd.collective_compute(
                "AllGather", mybir.AluOpType.bypass,
                replica_groups=[list(range(NCORES))],
                ins=[t2own[:]], outs=[t2full[:]],
            )

            # -------- layer-2: reuse L1 pool tags (same SBUF budget) -------
            sh2r = load(consts, nc, sh2_p, [P, D2], f32, "sh2r")
            fcwr = load(consts, nc, fcw_p, [P, D2], f32, "fcwr")
            g2w_t = load(consts, nc, g2w_p, [P, NB * T2g * 8], i16, "g1w")
            dd2w_t = load(consts, nc, dd2w_p, [P, NB * T2g * 8], i16, "dd1w")
            dloc_t = load(consts, nc, dloc2_p, [P, NB * T2g], bf16, "dloc1")

            T = T2g
            for b in range(NB):
                G = gp3.tile([P, T * TW2], bf16, tag="G")
                gathers(nc, G, g2w_t, b * T * 8, TA2, t2full[:], TW2)
                doneB = 0
                while doneB < TB2:
                    nt = min(TB2 - doneB, GNT)
                    K = nt * P
                    out_ap = apx(G[:], (TA2 + doneB) * TW2, [[TW2, nt], [1, TW2]])
                    nc.gpsimd.dma_gather(
                        out_ap=out_ap, in_ap=t2full[SPLIT:NCORES * NOP, :],
                        idxs_ap=g2w_t[:, (b * T + TA2 + doneB) * 8:(b * T + TA2 + doneB + nt) * 8],
                        num_idxs=K, num_idxs_reg=K, elem_size=TW2, queue_num=nextq())
                    doneB += nt
                if b % 2 == 0:
                    npair = 2 * T if b + 1 < NB else T
                    DdP = sbuf.tile([P, 2 * T * DW], f32, tag="Dd")
                    gathers(nc, DdP, dd2w_t, b * T * 8, npair, d2t[:], DW)
                Dd = DdP[:, (b % 2) * T * DW:(b % 2 + 1) * T * DW]
                me = gp3.tile([P, T * P], bf16, tag="me")
                nc.vector.tensor_tensor(
                    out=apx(me[:], 0, [[P, T], [1, P]]),
                    in0=apx(dloc_t[:], b * T, [[1, T], [0, P]]),
                    in1=apx(iota_r[:], 0, [[0, T], [1, P]]),
                    op=OP.is_equal)
                st = sbuf.tile([P, T * H], f32, tag="st_2")
                nc.vector.tensor_tensor(
                    out=apx(st[:], 0, [[H, T], [1, H]]),
                    in0=apx(G[:], D2, [[TW2, T], [1, H]]),
                    in1=apx(Dd, 0, [[DW, T], [1, H]]), op=OP.add)
                st2 = sbuf.tile([P, T * H], f32, tag="st2_2")
                nc.vector.tensor_scalar_mul(st2[:], st[:], 0.2)
                nc.vector.tensor_tensor(out=st[:], in0=st[:], in1=st2[:], op=OP.max)
                ex = sbuf.tile([P, T * H], bf16, tag="ex2")
                nc.scalar.activation(ex[:], st[:], AF.Exp)
                we = gp3.tile([P, T * R2], bf16, tag="we")
                nc.vector.tensor_tensor(
                    out=apx(we[:], 0, [[R2, T], [1, D2]]),
                    in0=apx(G[:], 0, [[TW2, T], [1, D2]]),
                    in1=apx(ex[:], 0, [[H, T], [1, H], [0, C2]]), op=OP.mult)
                nc.vector.tensor_copy(
                    out=apx(we[:], D2, [[R2, T], [1, H]]),
                    in_=apx(ex[:], 0, [[H, T], [1, H]]))
                po = psB.tile([P, R2], f32, tag="po")
                for t in range(T):
                    nc.tensor.matmul(
                        out=po[:],
                        lhsT=me[:, t * P:(t + 1) * P],
                        rhs=we[:, t * R2:(t + 1) * R2],
                        start=(t == 0), stop=(t == T - 1))
                rden = sbuf.tile([P, H], f32, tag="rden2")
                nc.vector.reciprocal(rden[:], po[:, D2:D2 + H])
                r = sbuf.tile([P, D2], f32, tag="r2")
                nc.vector.tensor_tensor(
                    out=apx(r[:], 0, [[C2, H], [1, C2]]),
                    in0=apx(po[:], 0, [[C2, H], [1, C2]]),
                    in1=apx(rden[:], 0, [[1, H], [0, C2]]), op=OP.mult)
                nc.vector.tensor_add(r[:], r[:], sh2r[:])
                tneg = sbuf.tile([P, D2], f32, tag="tneg2")
                nc.vector.tensor_scalar_min(tneg[:], r[:], 0.0)
                texp = sbuf.tile([P, D2], f32, tag="texp2")
                nc.scalar.activation(texp[:], tneg[:], AF.Exp)
                u = sbuf.tile([P, D2], f32, tag="u2")
                nc.vector.tensor_scalar_max(u[:], r[:], 0.0)
                nc.vector.tensor_add(u[:], u[:], texp[:])
                prodt = sbuf.tile([P, D2], f32, tag="prodt")
                nc.vector.tensor_tensor(out=prodt[:], in0=u[:], in1=fcwr[:],
                                        op=OP.mult)
                red = sbuf.tile([P, 1], f32, tag="red")
                nc.vector.tensor_reduce(red[:], prodt[:],
                                        axis=mybir.AxisListType.X, op=OP.add)
                orow = sbuf.tile([P, 1], f32, tag="orow")
                nc.vector.tensor_scalar_add(orow[:], red[:], float(cfc))
                nc.sync.dma_start(out=out_p[b * P:(b + 1) * P, :], in_=orow[:])

    nc.finalize()
    return nc


def run_spmd(nc, in_maps):
    from concourse.bass_utils import run_bass_kernel_spmd
    res = run_bass_kernel_spmd(nc, in_maps, core_ids=list(range(NCORES)))
    return res.results


def kernel(**inputs):
    in_maps, Ts, cfc = preprocess(**inputs)
    nc = build_module(Ts, cfc, reps=1)
    results = run_spmd(nc, in_maps)
    out = np.empty((N, 1), np.float32)
    for c in range(NCORES):
        out[c * NO:(c + 1) * NO] = results[c]["out"][:NO]
    return out



# revision 20
# speedup vs baseline: 1.0239x; 1.0239x over previous
"""Distributed 2-layer GAT (BangaloreGAT) on 8 TRN2 NeuronCores.

Strategy: partition destination nodes across the 8 cores (6250 each). Host
routes every edge (incl. self-loops) to the core owning its dst, sorts by
dst, and packs edges into 49 dst-blocks x T tiles of 128 slots. Per layer
each core builds an fp16 gather table row [h | s] in its own DRAM (layer 1:
replicated compute from x; layer 2: own stripe + AllGather), then per
dst-block: batched indirect-DMA gathers of the edge sources' table rows
(1024 idx/call — the SWDGE ucode cap — spread round-robin over 4 SWDGE
queues so descriptor-ring prep/drain pipelines instead of serializing),
an indicator-matrix build (is_equal vs iota, fp16), and TensorE fp16
matmuls that segment-reduce alpha-weighted features and softmax
denominators in f32 PSUM. Per-edge dst terms come from a separate f32
d-table gather (merged across block pairs to halve SWDGE launches).
BatchNorm/bias are folded host-side into the table weights (W_ext also
carries the attention dot-product columns); ELU's "-1" is folded into the
next layer's bias columns. The softmax/ELU/output chains stay f32 — the
final fc reduction amplifies operand noise ~7x, so bf16 there fails the
2e-2 gate while fp16 tables + f32 post-chains hold rel err at 4.3e-3.
"""
import sys
from contextlib import ExitStack
import numpy as np
import ml_dtypes

sys.path.insert(0, '/opt/trn_rl_repo')
sys.path.insert(0, '/root/problem')

# ---------------- problem constants (hardcoded from the spec) --------------
N = 50000
E = 800000
FIN = 128
H = 4
C1 = 64
C2 = 32
D1 = H * C1            # 256
D2 = H * C2            # 128
R1 = D1 + H            # 260 table-1 row (h|s)
R2 = D2 + H            # 132 table-2 row
NCORES = 8
NO = N // NCORES       # 6250 owned dst nodes / core
P = 128
NB = 49                # dst blocks per core: ceil(6250/128)
NOP = NB * P           # padded own nodes (6272)
NPAD = ((N + P - 1) // P) * P + P  # padded table-1 rows (50048+)
EPS_BN = 1e-5


def _multiwait_fix(nc):
    from concourse import mybir
    cnt = [0]
    for f in nc.m.functions:
        for bb in f.blocks:
            insts = bb.instructions
            new = []
            changed = False
            for inst in insts:
                si = getattr(inst, "sync_info", None)
                waits = list(si.on_wait) if si is not None else []
                if len(waits) > 1:
                    changed = True
                    for w in waits[:-1]:
                        cnt[0] += 1
                        nop = mybir.InstNoOp(name=f"I-ws{cnt[0]}", ins=[], outs=[])
                        nop.engine = inst.engine
                        nop.sync_info = mybir.SyncInfo(on_wait=[w], on_update=[])
                        new.append(nop)
                    si.on_wait = [waits[-1]]
                new.append(inst)
            if changed:
                bb.instructions = new


# ---------------------------- host preprocessing ---------------------------
def preprocess(x, edge_index, W1, a1_src, a1_dst, b1, g1, be1, m1, v1,
               W2, a2_src, a2_dst, b2, g2, be2, m2, v2, fcW, fcb):
    x = np.asarray(x, np.float32)
    ei = np.asarray(edge_index)
    src = np.concatenate([ei[0], np.arange(N, dtype=np.int64)]).astype(np.int64)
    dst = np.concatenate([ei[1], np.arange(N, dtype=np.int64)]).astype(np.int64)

    # folded weights
    W1 = np.asarray(W1, np.float32); W2 = np.asarray(W2, np.float32)
    a1_src = np.asarray(a1_src, np.float32); a1_dst = np.asarray(a1_dst, np.float32)
    a2_src = np.asarray(a2_src, np.float32); a2_dst = np.asarray(a2_dst, np.float32)
    g1 = np.asarray(g1, np.float32); be1 = np.asarray(be1, np.float32)
    m1 = np.asarray(m1, np.float32); v1 = np.asarray(v1, np.float32)
    g2 = np.asarray(g2, np.float32); be2 = np.asarray(be2, np.float32)
    m2 = np.asarray(m2, np.float32); v2 = np.asarray(v2, np.float32)
    b1 = np.asarray(b1, np.float32); b2 = np.asarray(b2, np.float32)
    fcW = np.asarray(fcW, np.float32); fcb = np.asarray(fcb, np.float32)

    scale1 = g1 / np.sqrt(v1 + EPS_BN)
    shift1 = be1 - m1 * scale1
    shtot1 = scale1 * b1 + shift1                      # [256]
    W1p = W1 * scale1[None, :]                         # [128,256]
    w_s1 = np.einsum('fhc,hc->fh', W1.reshape(FIN, H, C1), a1_src)  # [128,4]
    w_d1 = np.einsum('fhc,hc->fh', W1.reshape(FIN, H, C1), a1_dst)
    W1ext = np.concatenate([W1p, w_s1, w_d1], axis=1).astype(np.float32)  # [128,264]

    scale2 = g2 / np.sqrt(v2 + EPS_BN)
    shift2 = be2 - m2 * scale2
    shtot2 = scale2 * b2 + shift2                      # [128]
    W2p = W2 * scale2[None, :]                         # [256,128]
    w_s2 = np.einsum('fhc,hc->fh', W2.reshape(D1, H, C2), a2_src)   # [256,4]
    w_d2 = np.einsum('fhc,hc->fh', W2.reshape(D1, H, C2), a2_dst)
    W2ext = np.concatenate([W2p, w_s2, w_d2], axis=1).astype(np.float32)  # [256,136]
    c2 = -W2ext.sum(axis=0).astype(np.float32)         # [136] (x2 = u-1 fold)
    cfc = np.float32(fcb[0] - fcW.sum())
    fcw_row = fcW.reshape(1, D2).astype(np.float32)

    # --- edge routing: per core, sort by dst, pack into blocks ---
    owner = dst // NO
    per_core = []
    maxT = 1
    for c in range(NCORES):
        m = owner == c
        s_c = src[m]
        dl = (dst[m] - c * NO).astype(np.int64)        # [0, 6250)
        order = np.argsort(dl, kind='stable')
        s_c = s_c[order]; dl = dl[order]
        blk = dl // P
        cnt = np.bincount(blk, minlength=NB)
        maxT = max(maxT, int(np.ceil(cnt.max() / P)))
        per_core.append((s_c, dl, blk, cnt))

    # per-layer A/B split (int16 gather limit 32768): recompute tile counts
    SPLIT = 32768
    def pack_layer(per_core, rowfn):
        # returns per-core slot arrays + (TA, TB)
        TA = TB = 1
        packed = []
        for c in range(NCORES):
            s_c, dl, blk, cnt = per_core[c]
            rows = rowfn(s_c, c)
            blocks = []
            start = 0
            for b in range(NB):
                n_b = int(cnt[b])
                sl = slice(start, start + n_b)
                r = rows[sl]; d = dl[sl]
                isa = r < SPLIT
                blocks.append((r[isa], d[isa], r[~isa], d[~isa]))
                TA = max(TA, int(np.ceil(len(blocks[-1][0]) / P)))
                TB = max(TB, int(np.ceil(len(blocks[-1][2]) / P)))
                start += n_b
            packed.append(blocks)
        return packed, TA, TB

    rf1 = lambda s, c: ((s - c * NO) % N).astype(np.int32)
    rf2 = lambda s, c: ((s // NO) * NOP + (s % NO)).astype(np.int32)
    packed1, TA1, TB1 = pack_layer(per_core, rf1)
    packed2, TA2, TB2 = pack_layer(per_core, rf2)
    T1g, T2g = TA1 + TB1, TA2 + TB2

    def build_arrays(blocks, TA, TB):
        T = TA + TB
        gidx = np.zeros((NB, T * P), np.int32)
        dloc = np.full((NB, T * P), 999.0, np.float32)
        for b, (ra, da, rb, db) in enumerate(blocks):
            na, nb_ = len(ra), len(rb)
            gidx[b, :na] = ra
            dloc[b, :na] = (da - b * P).astype(np.float32)
            off = TA * P
            gidx[b, off:off + nb_] = rb - SPLIT
            dloc[b, off:off + nb_] = (db - b * P).astype(np.float32)
        ddid = np.where(dloc < 999.0, 0, 0).astype(np.int32)  # placeholder
        # d-table index = absolute local dst = dloc + 128*b (pad -> 0)
        ddid = np.zeros((NB, T * P), np.int32)
        for b in range(NB):
            v = dloc[b]
            ddid[b] = np.where(v < 999.0, v + b * P, 0).astype(np.int32)
        return gidx, dloc, ddid

    def wrap16(a):      # [NB, K] int -> wrapped int16 [128, NB*K//16]
        NBb, K = a.shape
        w = a.reshape(NBb, K // 16, 16).transpose(2, 0, 1).reshape(16, NBb * (K // 16))
        return np.tile(w, (8, 1)).astype(np.int16)

    def pm(a, T):
        return np.ascontiguousarray(
            a.reshape(NB, T, P).transpose(2, 0, 1).reshape(P, NB * T))

    in_maps = []
    for c in range(NCORES):
        g1, dl1, dd1 = build_arrays(packed1[c], TA1, TB1)
        g2, dl2, dd2 = build_arrays(packed2[c], TA2, TB2)
        xr = np.roll(x, -c * NO, axis=0)
        xT = np.zeros((FIN, NPAD), np.float32)
        xT[:, :N] = xr.T
        in_maps.append({
            "xT": xT.astype(np.float16),
            "g1w": wrap16(g1), "g2w": wrap16(g2),
            "dd1w": wrap16(dd1), "dd2w": wrap16(dd2),
            "dloc1": pm(dl1, T1g).astype(np.float16),
            "dloc2": pm(dl2, T2g).astype(np.float16),
            "W1ext": W1ext.astype(np.float16),
            "W2ext": W2ext.astype(np.float16),
            "sh1": np.tile(shtot1.reshape(1, D1), (P, 1)),
            "sh2": np.tile(shtot2.reshape(1, D2), (P, 1)),
            "c2": np.tile(c2.reshape(1, D2 + 2 * H), (P, 1)),
            "fcw": np.tile(fcw_row, (P, 1)),
        })
    return in_maps, (TA1, TB1, TA2, TB2), cfc
    return in_maps, T, cfc


# ------------------------------ bass builder -------------------------------
SPLIT = 32768
TW1 = 384   # table-1 row stride (fp16 elems, 768B)
TW2 = 256   # table-2 row stride (fp16 elems, 512B)
DW = 64     # d-table row stride (f32, 256B)
GNT = 8     # tiles (x128 idx) per dma_gather call (1024-idx ucode cap)


def build_module(Ts, cfc, reps=1, debug=False):
    from concourse import bass, mybir, bacc
    import concourse.tile as tile
    from concourse.masks import make_identity

    TA1, TB1, TA2, TB2 = Ts
    T1g, T2g = TA1 + TB1, TA2 + TB2
    f32 = mybir.dt.float32
    f32r = mybir.dt.float32r
    bf16 = mybir.dt.float16
    i16 = mybir.dt.int16
    AF = mybir.ActivationFunctionType
    OP = mybir.AluOpType

    nc = bacc.Bacc(dynamic_dma_scratch_size=32768, num_swdge_queues=4)
    xT_p = nc.declare_dram_parameter("xT", [FIN, NPAD], bf16, isOutput=False)
    g1w_p = nc.declare_dram_parameter("g1w", [P, NB * T1g * 8], i16, isOutput=False)
    g2w_p = nc.declare_dram_parameter("g2w", [P, NB * T2g * 8], i16, isOutput=False)
    dd1w_p = nc.declare_dram_parameter("dd1w", [P, NB * T1g * 8], i16, isOutput=False)
    dd2w_p = nc.declare_dram_parameter("dd2w", [P, NB * T2g * 8], i16, isOutput=False)
    dloc1_p = nc.declare_dram_parameter("dloc1", [P, NB * T1g], bf16, isOutput=False)
    dloc2_p = nc.declare_dram_parameter("dloc2", [P, NB * T2g], bf16, isOutput=False)
    W1e_p = nc.declare_dram_parameter("W1ext", [FIN, D1 + 2 * H], bf16, isOutput=False)
    W2e_p = nc.declare_dram_parameter("W2ext", [D1, D2 + 2 * H], bf16, isOutput=False)
    sh1_p = nc.declare_dram_parameter("sh1", [P, D1], f32, isOutput=False)
    sh2_p = nc.declare_dram_parameter("sh2", [P, D2], f32, isOutput=False)
    c2_p = nc.declare_dram_parameter("c2", [P, D2 + 2 * H], f32, isOutput=False)
    fcw_p = nc.declare_dram_parameter("fcw", [P, D2], f32, isOutput=False)
    out_p = nc.declare_dram_parameter("out", [NOP, 1], f32, isOutput=True)

    t1a = nc.dram_tensor("t1taba", [SPLIT, TW1], bf16)
    t1b = nc.dram_tensor("t1tabb", [NPAD - SPLIT, TW1], bf16)
    d1t = nc.dram_tensor("d1tab", [8192, DW], f32)
    t2own = nc.dram_tensor("t2own", [NOP, TW2], bf16)
    d2t = nc.dram_tensor("d2tab", [NOP, DW], f32)
    t2full = nc.dram_tensor("t2full", [NCORES * NOP, TW2], bf16, addr_space="Shared")

    def apx(base_ap, off, pattern):
        return bass.AP(tensor=base_ap.tensor, offset=base_ap.offset + off,
                       ap=[list(base_ap.ap[0])] + [list(q) for q in pattern])

    def load(tc_pool, nc, param, shape, dt, tag):
        t = tc_pool.tile(shape, dt, tag=tag)
        nc.sync.dma_start(out=t[:], in_=param[:])
        return t

    QCTR = [0]

    def nextq():
        QCTR[0] = (QCTR[0] + 1) % 4
        return QCTR[0]

    def gathers(nc, dest, widx_t, wcol0, n_tiles, tbl_ap, elem):
        # chunked dma_gather: <=1024 idx per call (SWDGE ring limit/queue)
        done = 0
        while done < n_tiles:
            nt = min(n_tiles - done, GNT)
            K = nt * P
            out_ap = apx(dest[:], (done) * elem, [[elem, nt], [1, elem]])
            nc.gpsimd.dma_gather(
                out_ap=out_ap, in_ap=tbl_ap,
                idxs_ap=widx_t[:, wcol0 + done * 8: wcol0 + (done + nt) * 8],
                num_idxs=K, num_idxs_reg=K, elem_size=elem, queue_num=nextq())
            done += nt

    NTILE1 = NPAD // P

    for rep in range(reps):
        with tile.TileContext(nc) as tc, ExitStack() as ctx:
            consts = ctx.enter_context(tc.tile_pool(name=f"consts{rep}", bufs=1))
            sbuf = ctx.enter_context(tc.tile_pool(name=f"sbuf{rep}", bufs=2))
            gp3 = ctx.enter_context(tc.tile_pool(name=f"gp3{rep}", bufs=3))
            sb3 = ctx.enter_context(tc.tile_pool(name=f"sb3{rep}", bufs=3))
            psA = ctx.enter_context(tc.tile_pool(name=f"psA{rep}", bufs=2, space="PSUM"))
            psB = ctx.enter_context(tc.tile_pool(name=f"psB{rep}", bufs=3, space="PSUM"))
            psC = ctx.enter_context(tc.tile_pool(name=f"psC{rep}", bufs=2, space="PSUM"))

            w1e = consts.tile([FIN, D1 + 2 * H], bf16)
            nc.sync.dma_start(out=w1e[:], in_=W1e_p[:])
            w2e = consts.tile([P, 2, D2 + 2 * H], bf16)
            nc.sync.dma_start(out=w2e[:, 0, :], in_=W2e_p[0:P, :])
            nc.sync.dma_start(out=w2e[:, 1, :], in_=W2e_p[P:2 * P, :])
            sh1r = load(consts, nc, sh1_p, [P, D1], f32, "sh1r")
            c2r = load(consts, nc, c2_p, [P, D2 + 2 * H], f32, "c2r")
            g1w_t = load(consts, nc, g1w_p, [P, NB * T1g * 8], i16, "g1w")
            dd1w_t = load(consts, nc, dd1w_p, [P, NB * T1g * 8], i16, "dd1w")
            dloc_t = load(consts, nc, dloc1_p, [P, NB * T1g], bf16, "dloc1")
            iota_i = consts.tile([P, P], mybir.dt.int32)
            nc.gpsimd.iota(iota_i[:], pattern=[[1, P]], base=0, channel_multiplier=0)
            iota_r = consts.tile([P, P], bf16)
            nc.vector.tensor_copy(iota_r[:], iota_i[:])
            ident = consts.tile([P, P], bf16)
            make_identity(nc, ident[:])

            # --- table-1 build (replicated; 8 tiles/iter; A region first) --
            W4 = 8
            JSPLIT = SPLIT // (W4 * P)  # iters 0..JSPLIT-1 -> t1a, rest -> t1b
            for j in range(NTILE1 // W4):
                xc = sb3.tile([FIN, W4 * P], bf16, tag="xc")
                nc.sync.dma_start(out=xc[:], in_=xT_p[:, j * W4 * P:(j + 1) * W4 * P])
                row = sb3.tile([P, W4, R1], bf16, tag="t1row")
                drow = sb3.tile([P, W4, H], f32, tag="t1d")
                for k in range(W4):
                    ps = psA.tile([P, D1 + 2 * H], f32, tag="t1ps")
                    nc.tensor.matmul(out=ps[:], lhsT=xc[:, k * P:(k + 1) * P],
                                     rhs=w1e[:], start=True, stop=True)
                    if k % 2 == 0:
                        nc.vector.tensor_copy(row[:, k, :], ps[:, 0:R1])
                    else:
                        nc.scalar.copy(row[:, k, :], ps[:, 0:R1])
                    if j * W4 + k < NB:
                        nc.vector.tensor_copy(drow[:, k, :], ps[:, R1:R1 + H])
                # one 3D DMA writes 8 consecutive row-tiles into its region
                tdst = t1a if j < JSPLIT else t1b
                joff = j if j < JSPLIT else j - JSPLIT
                tref = tdst[0:P, 0:R1]
                nc.sync.dma_start(
                    out=bass.AP(tensor=tref.tensor, offset=joff * W4 * P * TW1,
                                ap=[[TW1, P], [P * TW1, W4], [1, R1]]),
                    in_=row[:])
                if j * W4 < NB:
                    d1b = d1t[0:P, 0:H]
                    nc.sync.dma_start(
                        out=bass.AP(tensor=d1b.tensor, offset=j * W4 * P * DW,
                                    ap=[[DW, P], [P * DW, W4], [1, H]]),
                        in_=drow[:])

            # ---------------- layer-1 edge pass ----------------------------
            T = T1g
            for b in range(NB):
                G = gp3.tile([P, T * TW1], bf16, tag="G")
                gathers(nc, G, g1w_t, b * T * 8, TA1, t1a[:], TW1)
                # B-half gather (table view offset SPLIT rows)
                doneB = 0
                while doneB < TB1:
                    nt = min(TB1 - doneB, GNT)
                    K = nt * P
                    out_ap = apx(G[:], (TA1 + doneB) * TW1, [[TW1, nt], [1, TW1]])
                    nc.gpsimd.dma_gather(
                        out_ap=out_ap, in_ap=t1b[:],
                        idxs_ap=g1w_t[:, (b * T + TA1 + doneB) * 8:(b * T + TA1 + doneB + nt) * 8],
                        num_idxs=K, num_idxs_reg=K, elem_size=TW1, queue_num=nextq())
                    doneB += nt
                if b % 2 == 0:
                    npair = 2 * T if b + 1 < NB else T
                    DdP = sbuf.tile([P, 2 * T * DW], f32, tag="Dd")
                    gathers(nc, DdP, dd1w_t, b * T * 8, npair, d1t[:], DW)
                Dd = DdP[:, (b % 2) * T * DW:(b % 2 + 1) * T * DW]
                me = gp3.tile([P, T * P], bf16, tag="me")
                nc.vector.tensor_tensor(
                    out=apx(me[:], 0, [[P, T], [1, P]]),
                    in0=apx(dloc_t[:], b * T, [[1, T], [0, P]]),
                    in1=apx(iota_r[:], 0, [[0, T], [1, P]]),
                    op=OP.is_equal)
                st = sbuf.tile([P, T * H], f32, tag="st")
                nc.vector.tensor_tensor(
                    out=apx(st[:], 0, [[H, T], [1, H]]),
                    in0=apx(G[:], D1, [[TW1, T], [1, H]]),
                    in1=apx(Dd, 0, [[DW, T], [1, H]]), op=OP.add)
                st2 = sbuf.tile([P, T * H], f32, tag="st2")
                nc.vector.tensor_scalar_mul(st2[:], st[:], 0.2)
                nc.vector.tensor_tensor(out=st[:], in0=st[:], in1=st2[:], op=OP.max)
                ex = sbuf.tile([P, T * H], bf16, tag="ex")
                nc.scalar.activation(ex[:], st[:], AF.Exp)
                we = gp3.tile([P, T * R1], bf16, tag="we")
                nc.vector.tensor_tensor(
                    out=apx(we[:], 0, [[R1, T], [1, D1]]),
                    in0=apx(G[:], 0, [[TW1, T], [1, D1]]),
                    in1=apx(ex[:], 0, [[H, T], [1, H], [0, C1]]), op=OP.mult)
                nc.vector.tensor_copy(
                    out=apx(we[:], D1, [[R1, T], [1, H]]),
                    in_=apx(ex[:], 0, [[H, T], [1, H]]))
                po = psB.tile([P, R1], f32, tag="po")
                for t in range(T):
                    nc.tensor.matmul(
                        out=po[:],
                        lhsT=me[:, t * P:(t + 1) * P],
                        rhs=we[:, t * R1:(t + 1) * R1],
                        start=(t == 0), stop=(t == T - 1))
                den = sbuf.tile([P, H], f32, tag="den")
                nc.vector.tensor_scalar_max(den[:], po[:, D1:D1 + H], 1e-30)
                rden = sbuf.tile([P, H], f32, tag="rden")
                nc.vector.reciprocal(rden[:], den[:])
                r = sbuf.tile([P, D1], f32, tag="r")
                nc.vector.tensor_tensor(
                    out=apx(r[:], 0, [[C1, H], [1, C1]]),
                    in0=apx(po[:], 0, [[C1, H], [1, C1]]),
                    in1=apx(rden[:], 0, [[1, H], [0, C1]]), op=OP.mult)
                nc.vector.tensor_add(r[:], r[:], sh1r[:])
                tneg = sbuf.tile([P, D1], f32, tag="tneg")
                nc.vector.tensor_scalar_min(tneg[:], r[:], 0.0)
                texp = sbuf.tile([P, D1], f32, tag="texp")
                nc.scalar.activation(texp[:], tneg[:], AF.Exp)
                u = sbuf.tile([P, D1], bf16, tag="u")
                nc.vector.tensor_scalar_max(u[:], r[:], 0.0)
                nc.vector.tensor_add(u[:], u[:], texp[:])
                uT = sbuf.tile([P, 2, P], bf16, tag="uT")
                for k in range(2):
                    pt = psC.tile([P, P], bf16, tag="pt", bufs=1)
                    nc.tensor.transpose(out=pt[:], in_=u[:, k * P:(k + 1) * P],
                                        identity=ident[:])
                    nc.vector.tensor_copy(uT[:, k, :], pt[:])
                p2 = psC.tile([P, D2 + 2 * H], f32, tag="p2")
                for k in range(2):
                    nc.tensor.matmul(out=p2[:], lhsT=uT[:, k, :],
                                     rhs=w2e[:, k, :],
                                     start=(k == 0), stop=(k == 1))
                row2 = sbuf.tile([P, R2], bf16, tag="row2")
                nc.vector.tensor_add(row2[:], p2[:, 0:R2], c2r[:, 0:R2])
                nc.sync.dma_start(out=t2own[b * P:(b + 1) * P, 0:R2],
                                  in_=row2[:])
                d2row = sbuf.tile([P, H], f32, tag="d2row")
                nc.vector.tensor_add(d2row[:], p2[:, R2:R2 + H], c2r[:, R2:R2 + H])
                nc.sync.dma_start(out=d2t[b * P:(b + 1) * P, 0:H],
                                  in_=d2row[:])

            # -------- AllGather inside the same TileContext ---------------
            nc.gpsimd.collective_compute(
                "AllGather", mybir.AluOpType.bypass,
                replica_groups=[list(range(NCORES))],
                ins=[t2own[:]], outs=[t2full[:]],
            )

            # -------- layer-2: reuse L1 pool tags (same SBUF budget) -------
            sh2r = load(consts, nc, sh2_p, [P, D2], f32, "sh2r")
            fcwr = load(consts, nc, fcw_p, [P, D2], f32, "fcwr")
            g2w_t = load(consts, nc, g2w_p, [P, NB * T2g * 8], i16, "g1w")
            dd2w_t = load(consts, nc, dd2w_p, [P, NB * T2g * 8], i16, "dd1w")
            dloc_t = load(consts, nc, dloc2_p, [P, NB * T2g], bf16, "dloc1")

            T = T2g
            for b in range(NB):
                G = gp3.tile([P, T * TW2], bf16, tag="G")
                gathers(nc, G, g2w_t, b * T * 8, TA2, t2full[:], TW2)
                doneB = 0
                while doneB < TB2:
                    nt = min(TB2 - doneB, GNT)
                    K = nt * P
                    out_ap = apx(G[:], (TA2 + doneB) * TW2, [[TW2, nt], [1, TW2]])
                    nc.gpsimd.dma_gather(
                        out_ap=out_ap, in_ap=t2full[SPLIT:NCORES * NOP, :],
                        idxs_ap=g2w_t[:, (b * T + TA2 + doneB) * 8:(b * T + TA2 + doneB + nt) * 8],
                        num_idxs=K, num_idxs_reg=K, elem_size=TW2, queue_num=nextq())
                    doneB += nt
                if b % 2 == 0:
                    npair = 2 * T if b + 1 < NB else T
                    DdP = sbuf.tile([P, 2 * T * DW], f32, tag="Dd")
                    gathers(nc, DdP, dd2w_t, b * T * 8, npair, d2t[:], DW)
                Dd = DdP[:, (b % 2) * T * DW:(b % 2 + 1) * T * DW]
                me = gp3.tile([P, T * P], bf16, tag="me")
                nc.vector.tensor_tensor(
                    out=apx(me[:], 0, [[P, T], [1, P]]),
                    in0=apx(dloc_t[:], b * T, [[1, T], [0, P]]),
                    in1=apx(iota_r[:], 0, [[0, T], [1, P]]),
                    op=OP.is_equal)
                st = sbuf.tile([P, T * H], f32, tag="st_2")
                nc.vector.tensor_tensor(
                    out=apx(st[:], 0, [[H, T], [1, H]]),
                    in0=apx(G[:], D2, [[TW2, T], [1, H]]),
                    in1=apx(Dd, 0, [[DW, T], [1, H]]), op=OP.add)
                st2 = sbuf.tile([P, T * H], f32, tag="st2_2")
                nc.vector.tensor_scalar_mul(st2[:], st[:], 0.2)
                nc.vector.tensor_tensor(out=st[:], in0=st[:], in1=st2[:], op=OP.max)
                ex = sbuf.tile([P, T * H], bf16, tag="ex2")
                nc.scalar.activation(ex[:], st[:], AF.Exp)
                we = gp3.tile([P, T * R2], bf16, tag="we")
                nc.vector.tensor_tensor(
                    out=apx(we[:], 0, [[R2, T], [1, D2]]),
                    in0=apx(G[:], 0, [[TW2, T], [1, D2]]),
                    in1=apx(ex[:], 0, [[H, T], [1, H], [0, C2]]), op=OP.mult)
                nc.vector.tensor_copy(
                    out=apx(we[:], D2, [[R2, T], [1, H]]),
                    in_=apx(ex[:], 0, [[H, T], [1, H]]))
                po = psB.tile([P, R2], f32, tag="po")
                for t in range(T):
                    nc.tensor.matmul(
                        out=po[:],
                        lhsT=me[:, t * P:(t + 1) * P],
                        rhs=we[:, t * R2:(t + 1) * R2],
                        start=(t == 0), stop=(t == T - 1))
                den = sbuf.tile([P, H], f32, tag="den2")
                nc.vector.tensor_scalar_max(den[:], po[:, D2:D2 + H], 1e-30)
                rden = sbuf.tile([P, H], f32, tag="rden2")
                nc.vector.reciprocal(rden[:], den[:])
                r = sbuf.tile([P, D2], f32, tag="r2")
                nc.vector.tensor_tensor(
                    out=apx(r[:], 0, [[C2, H], [1, C2]]),
                    in0=apx(po[:], 0, [[C2, H], [1, C2]]),
                    in1=apx(rden[:], 0, [[1, H], [0, C2]]), op=OP.mult)
                nc.vector.tensor_add(r[:], r[:], sh2r[:])
                tneg = sbuf.tile([P, D2], f32, tag="tneg2")
                nc.vector.tensor_scalar_min(tneg[:], r[:], 0.0)
                texp = sbuf.tile([P, D2], f32, tag="texp2")
                nc.scalar.activation(texp[:], tneg[:], AF.Exp)
                u = sbuf.tile([P, D2], f32, tag="u2")
                nc.vector.tensor_scalar_max(u[:], r[:], 0.0)
                nc.vector.tensor_add(u[:], u[:], texp[:])
                prodt = sbuf.tile([P, D2], f32, tag="prodt")
                nc.vector.tensor_tensor(out=prodt[:], in0=u[:], in1=fcwr[:],
                                        op=OP.mult)
                red = sbuf.tile([P, 1], f32, tag="red")
                nc.vector.tensor_reduce(red[:], prodt[:],
                                        axis=mybir.AxisListType.X, op=OP.add)
                orow = sbuf.tile([P, 1], f32, tag="orow")
                nc.vector.tensor_scalar_add(orow[:], red[:], float(cfc))
                nc.sync.dma_start(out=out_p[b * P:(b + 1) * P, :], in_=orow[:])

    nc.finalize()
    return nc


def run_spmd(nc, in_maps):
    from concourse.bass_utils import run_bass_kernel_spmd
    res = run_bass_kernel_spmd(nc, in_maps, core_ids=list(range(NCORES)))
    return res.results


def kernel(**inputs):
    in_maps, Ts, cfc = preprocess(**inputs)
    nc = build_module(Ts, cfc, reps=1)
    results = run_spmd(nc, in_maps)
    out = np.empty((N, 1), np.float32)
    for c in range(NCORES):
        out[c * NO:(c + 1) * NO] = results[c]["out"][:NO]
    return out

